# revision 56
# baseline (speedup 1.0000x reference)
"""Trainium2 Bass kernel for nn_AttentionModel (4-layer dense transformer).

Contract: kernel(**inputs) takes FULL unsharded inputs (as produced by
setup_inputs) and returns the FULL output [N, L, V] fp32.

Sharding: data-parallel over batch N=8 across the 8 NeuronCores — each core
runs the complete transformer for one batch element (identical NEFF, per-core
tokens). No collectives needed; the host stacks the per-core outputs.

Numerics: all matmul operands are fp16 (weights converted host-side, the
residual stream and attention tensors on-device). PSUM accumulation is fp32,
LN statistics fp32. Measured end-to-end max-normalized rel err ~5e-4 vs the
fp32 reference (gate is 2e-2). fp16 operands give the same 1 cycle/col PE
rate as float32r but without the >=256-column restriction, make PE transposes
1 cyc/row instead of 2, and halve SBUF/DMA traffic.

Per-core dataflow (L=1024, F=512, H=8, KD=QD=64, NL=4, V=1024):
  - embedding: indirect-DMA gather of fp16 embed rows by token -> x0 [L, F]
  - activations in two layouts: natural [l(128-part) x F] for LN/residual/
    attend-normalize; T [F(128-part) x L] as matmul operands. PE transposes
    convert; the 4 per-chunk transposes of one l-block share one PSUM bank and
    are drained by a single DVE copy (fp16 2x mode).
  - per layer:
      kT = Wk^T x^T, vT = Wv^T x^T (psum->sbuf copies on ACT, which is idle
        at layer start), q = x Wq stored [j-chunk, head, 65] with a ones
        column so the attend matmul also produces softmax row-sums (copy on
        DVE)
      scores^T[j,i] = v k^T per head on disjoint PE row-groups per head pair.
        Per (pair, jc): jc<4 uses a [128,1024] 2-bank psum tile per head
        (full i span, one exp); jc>=4 packs both heads into one [128,2,512]
        tile (one 2D-AP exp for both heads). This batching cuts ACT exp
        instruction count ~2x vs per-512-col exps.
      att_u = exp(scores^T - 5) fp16 (the -5 keeps exp in fp16 range and
        cancels in the softmax ratio); diagonal tiles triangle-zeroed with
        gpsimd affine_select (keep j<=i)
      x_new[i-block, head pair] = att_u^T @ [q | 1] (fp16 matmuls, one psum
        bank per pair): col 64 of each head = softmax row-sum; one strided
        reciprocal + one 0-stride-broadcast multiply normalizes during the
        psum->sbuf copy
      x_newT via PE transposes; MLP h1T = relu(W1^T x_newT) on DVE
        (or ACT activation when b1 != 0); h = h1T^T W2; y = LN(x + h)
        (bn_stats/bn_aggr per chunk, rstd = exp(-0.5 ln(var+eps)) on ACT);
        yT via PE transposes
  - unembed: logits = x4 Wout (+ bout), psum->staging copies alternate
    ACT/DVE, DMA'd out per [128, 512] fp32 tile.
"""

import numpy as np

import concourse.bass as bass
import concourse.mybir as mybir
import concourse.tile as tile
from concourse import bacc
from concourse.bass_utils import run_bass_kernel_spmd
from concourse.masks import make_identity

# Model dims (hardcoded per the problem spec)
V, F, NL, H, KD, QD = 1024, 512, 4, 8, 64, 64
N, L = 8, 1024
HQ = H * QD  # 512
P = 128
FC = F // P      # 4 f-chunks
LB = L // P      # 8 l-blocks of 128
NCORES = 8

f32 = mybir.dt.float32
f16 = mybir.dt.float16
i32 = mybir.dt.int32
AF = mybir.ActivationFunctionType
OP = mybir.AluOpType

_NC_CACHE: dict = {}
ABLATE_EXPQ = False  # timing experiment: exp only ~1/4 of score columns
ABLATE_ATT = False   # timing experiment: attend accumulates only 2 jc steps
ABLATE_SC = False    # timing experiment: skip jc<4 bank-1 score matmuls
ABLATE_LN = False    # timing experiment: skip LN stats/apply (residual only)
ABLATE_OUT = False   # timing experiment: skip output DMAs
EXPP_BUFS = 28   # in-flight [128,1024] fp16 att tiles (score->exp->attend
# pipeline depth); 12 tiles per head pair, so ~2 pairs of lookahead + margin
WP_BUFS = 12     # fp16 weight tiles; 5/layer -> ~2.4 layers of prefetch
LOOKAHEAD = 1    # score pairs emitted ahead of the attend consuming them
# (2 measured worse: 481us vs 428us — SBUF pressure + lost kv interleave)
PSUM_CFG = (2, 2, 2)  # bufs for (ps scores 2-bank, pp 1-bank, pa 1-bank)
# bank budget: 2*2 + 2 + 2 = 8

# fp16 DRAM parameters (host converts); everything else fp32/int32
_F16_PARAMS = ("embed", "Wq", "Wk", "Wv", "W1", "W2", "Wout")


class _Bacc(bacc.Bacc):
    """Bacc with activation-table-set selection pinned to
    natural_log_exp_and_others (contains Exp, Ln, Relu, Copy — everything this
    kernel uses) so the load-insertion pass emits one table load instead of
    thrashing between per-function sets (~2.7us per swap)."""

    def insert_act_table_loads(self):
        from concourse.hw_specs import get_activation_tables
        import concourse.mybir as _mb

        has_activation = any(
            isinstance(i, _mb.InstActivation)
            for b in self.main_func.blocks
            for i in b.instructions
        )
        if not has_activation:
            return
        keep = {AF.Exp, AF.Ln, AF.Relu, AF.Copy}
        chosen = "natural_log_exp_and_others"
        full = get_activation_tables(self.m.arch)
        assert keep <= full[chosen], (chosen, keep - full[chosen])
        tables = [
            (name, (fns if name == chosen else fns - keep))
            for name, fns in full.items()
        ]
        import bass_rust as _bass_rust
        _bass_rust.insert_act_table_loads(self, tables)


def _ln_apply(nc, y, b, mv8, rstd8, use_gamma, use_beta, gamma_b, beta_b):
    t = y[:, b, :]
    nc.vector.tensor_scalar(
        t, t, mv8[:, b, 0:1], rstd8[:, b:b + 1],
        op0=OP.subtract, op1=OP.mult)
    if use_gamma:
        nc.vector.tensor_mul(t, t, gamma_b[:])
    if use_beta:
        nc.vector.tensor_add(t, t, beta_b[:])


PHASE_MARKS: list = []  # (label, first_instruction_number) for simprof


def _build(flags, repeat=1):
    use_b1, use_b2, use_gamma, use_beta, use_bout = flags
    nc = _Bacc("TRN2", target_bir_lowering=False, debug=False,
               num_devices=NCORES)
    PHASE_MARKS.clear()

    def mark(label):
        nm = nc.get_next_instruction_name()  # burns one name; fine
        PHASE_MARKS.append((label, int(nm.split("-")[1])))

    tokens = nc.declare_dram_parameter("tokens", [L], i32, isOutput=False)
    embed = nc.declare_dram_parameter("embed", [V, F], f16, isOutput=False)
    Wq = nc.declare_dram_parameter("Wq", [NL, F, HQ], f16, isOutput=False)
    Wk = nc.declare_dram_parameter("Wk", [NL, F, H * KD], f16, isOutput=False)
    Wv = nc.declare_dram_parameter("Wv", [NL, F, H * KD], f16, isOutput=False)
    W1 = nc.declare_dram_parameter("W1", [NL, HQ, F], f16, isOutput=False)
    b1 = nc.declare_dram_parameter("b1", [NL, F], f32, isOutput=False)
    W2 = nc.declare_dram_parameter("W2", [NL, F, F], f16, isOutput=False)
    b2 = nc.declare_dram_parameter("b2", [NL, F], f32, isOutput=False)
    gamma = nc.declare_dram_parameter("gamma", [NL, F], f32, isOutput=False)
    beta = nc.declare_dram_parameter("beta", [NL, F], f32, isOutput=False)
    Wout = nc.declare_dram_parameter("Wout", [F, V], f16, isOutput=False)
    bout = nc.declare_dram_parameter("bout", [V], f32, isOutput=False)
    out = nc.declare_dram_parameter("out", [L, V], f32, isOutput=True)

    with tile.TileContext(nc) as tc:
        with (
            tc.tile_pool(name="bigT", bufs=5) as bigT,    # [P, FC, L] f16
            tc.tile_pool(name="nat", bufs=3) as natp,     # [P, LB, F] f16
            tc.tile_pool(name="qp", bufs=1) as qp,        # [P, LB, H, 65] f16
            tc.tile_pool(name="expp", bufs=EXPP_BUFS) as expp,  # [P, 1024] f16
            tc.tile_pool(name="wp", bufs=WP_BUFS) as wp,  # [P, FC, 512] f16
            tc.tile_pool(name="cst", bufs=1) as cst,
            tc.tile_pool(name="sm", bufs=16) as sm,       # small scalars
            tc.tile_pool(name="op", bufs=4) as outp,      # [P, 512] f32 staging
            tc.tile_pool(name="ps", bufs=PSUM_CFG[0], space="PSUM") as psp,
            tc.tile_pool(name="pp", bufs=PSUM_CFG[1], space="PSUM") as pp,
            tc.tile_pool(name="pa", bufs=PSUM_CFG[2], space="PSUM") as pa,
        ):
            # ---- constants ----
            ident = cst.tile([P, P], f16, tag="ident")
            make_identity(nc, ident[:])
            eps_t = cst.tile([P, 1], f32, tag="eps")
            nc.vector.memset(eps_t[:], 1e-5)
            neg5_t = cst.tile([P, 1], f32, tag="neg5")
            nc.vector.memset(neg5_t[:], -5.0)
            if use_b1:
                b1_sb = cst.tile([P, NL, FC], f32, tag="b1")
                nc.sync.dma_start(b1_sb[:], b1.rearrange("l (c p) -> p l c", p=P))
            if use_bout:
                bout_b = cst.tile([P, V], f32, tag="bout")
                bout_ap = bout[:]
                nc.sync.dma_start(
                    bout_b[:],
                    bass.AP(tensor=bout_ap.tensor, offset=bout_ap.offset,
                            ap=[[0, P]] + bout_ap.ap),
                )

            def bcast_row(dram_row_ap, tag):
                t = cst.tile([P, F], f32, tag=tag)
                nc.sync.dma_start(
                    t[:],
                    bass.AP(tensor=dram_row_ap.tensor, offset=dram_row_ap.offset,
                            ap=[[0, P]] + dram_row_ap.ap),
                )
                return t

            import contextlib
            _loop = (tc.For_i(0, repeat, 1) if repeat > 1
                     else contextlib.nullcontext())
            with _loop:
                # ---- embedding gather ----
                mark("embed")
                tok_sb = cst.tile([P, LB], i32, tag="tok")
                nc.sync.dma_start(tok_sb[:], tokens.rearrange("(b p) -> p b", p=P))
                x_nat = natp.tile([P, LB, F], f16, tag="nat")
                for b in range(LB):
                    nc.gpsimd.indirect_dma_start(
                        out=x_nat[:, b, :], out_offset=None,
                        in_=embed[:],
                        in_offset=bass.IndirectOffsetOnAxis(
                            ap=tok_sb[:, b:b + 1], axis=0),
                    )

                def transpose_b(src_nat, dst_T, bs):
                    """Transpose l-blocks bs of src natural [P, LB, F] f16
                    into dst T [P, FC, L] f16. 4 per-chunk PE transposes share
                    one fp16 psum tile (half a bank), drained by a single
                    512-element DVE copy per l-block (fp16 2x mode)."""
                    for b in bs:
                        # shares the "pp" tag (and so its psum slots) with the
                        # matmul psum tiles — psum pool slots are per-tag
                        tp = pp.tile([P, 512], f16, tag="pp")
                        for c in range(FC):
                            nc.tensor.transpose(
                                tp[:, c * P:(c + 1) * P],
                                src_nat[:, b, c * P:(c + 1) * P], ident[:])
                        nc.vector.tensor_copy(
                            dst_T[:, :, b * P:(b + 1) * P],
                            tp[:].rearrange("p (c q) -> p c q", c=FC))

                def load_weights(li):
                    wq_t = wp.tile([P, FC, HQ], f16, tag="w")
                    wk_t = wp.tile([P, FC, HQ], f16, tag="w")
                    wv_t = wp.tile([P, FC, HQ], f16, tag="w")
                    nc.sync.dma_start(wq_t[:], Wq[li].rearrange("(c p) o -> p c o", p=P))
                    nc.sync.dma_start(wk_t[:], Wk[li].rearrange("(c p) o -> p c o", p=P))
                    nc.sync.dma_start(wv_t[:], Wv[li].rearrange("(c p) o -> p c o", p=P))
                    w1_t = wp.tile([P, FC, F], f16, tag="w")
                    w2_t = wp.tile([P, FC, F], f16, tag="w")
                    nc.sync.dma_start(w1_t[:], W1[li].rearrange("(c p) o -> p c o", p=P))
                    nc.sync.dma_start(w2_t[:], W2[li].rearrange("(c p) o -> p c o", p=P))
                    return dict(wq=wq_t, wk=wk_t, wv=wv_t, w1=w1_t, w2=w2_t)

                def new_attn_state():
                    kT = bigT.tile([P, FC, L], f16, tag="bigT")
                    vT = bigT.tile([P, FC, L], f16, tag="bigT")
                    q = qp.tile([P, LB, H, 65], f16, tag="q")
                    return dict(kT=kT, vT=vT, q=q, store={})

                def emit_kv(W, S, xT_, oc, lc):
                    # kT/vT chunk oc, l-half lc. Copies for oc 0 go on ACT
                    # (no exps pending there yet); later chunks drain on DVE
                    # so the ACT queue stays clear for exps.
                    for w_t, oT in ((W["wk"], S["kT"]), (W["wv"], S["vT"])):
                        ps = pp.tile([P, 512], f32, tag="pp")
                        for fc in range(FC):
                            nc.tensor.matmul(
                                ps[:],
                                w_t[:, fc, oc * P:(oc + 1) * P],
                                xT_[:, fc, lc * 512:(lc + 1) * 512],
                                start=(fc == 0), stop=(fc == FC - 1))
                        dst = oT[:, oc, lc * 512:(lc + 1) * 512]
                        nc.vector.tensor_copy(dst, ps[:])

                def emit_q(W, S, xT_, bs):
                    # q natural fp16, [P(j), jc, head, 65] with a trailing
                    # ones column so attend also yields softmax row-sums;
                    # psum->sbuf on ACT (still idle pre-exp)
                    q_sb = S["q"]
                    if bs[0] == 0:
                        nc.vector.memset(q_sb[:, :, :, 64:65], 1.0)
                    for b in bs:
                        ps = pp.tile([P, 512], f32, tag="pp")
                        for fc in range(FC):
                            nc.tensor.matmul(
                                ps[:], xT_[:, fc, b * P:(b + 1) * P],
                                W["wq"][:, fc, :],
                                start=(fc == 0), stop=(fc == FC - 1))
                        nc.vector.tensor_copy(
                            q_sb[:, b, :, 0:64],
                            ps[:].rearrange("p (h d) -> p h d", h=H))

                xT = bigT.tile([P, FC, L], f16, tag="bigT")
                mark("embedT")
                transpose_b(x_nat, xT, range(LB))

                # head for layer 0 (later layers' heads are emitted inside
                # the previous layer's tail, interleaved with the LN chain)
                W = load_weights(0)
                S = new_attn_state()
                mark("L0.kv")
                emit_kv(W, S, xT, 0, 0)
                emit_kv(W, S, xT, 0, 1)
                mark("L0.q")
                emit_q(W, S, xT, range(LB))

                # attention, processed in head pairs so the K=64 score
                # matmuls land on disjoint PE row-groups (partition bases
                # 0/64) and back-to-back weight loads overlap
                def emit_scores(S, hpair):
                        kT, vT = S["kT"], S["vT"]
                        exp_store = S["store"]
                        heads = (2 * hpair, 2 * hpair + 1)
                        hc = hpair
                        tiles = {}   # (h, jc) -> (exp_tile, iofs) where the
                        # attend lhsT for i-block b is et[:, iofs + b*128 :]
                        for jc in range(LB):
                            if jc < 4:
                                # one [128,1024] (2-bank) tile per head, full
                                # i span, single exp instruction
                                for h in heads:
                                    hb = 64 * (h % 2)
                                    st = vT[hb:hb + KD, hc, jc * P:(jc + 1) * P]
                                    ps = psp.tile([P, 1024], f32, tag="ps")
                                    nc.tensor.matmul(
                                        ps[:, jc * P:512], st,
                                        kT[hb:hb + KD, hc, jc * P:512],
                                        start=True, stop=True)
                                    if not ABLATE_SC:
                                        nc.tensor.matmul(
                                            ps[:, 512:1024], st,
                                            kT[hb:hb + KD, hc, 512:1024],
                                            start=True, stop=True)
                                    et = expp.tile([P, 1024], f16, tag="exp")
                                    # bias=-5: softmax is shift-invariant
                                    # (numerator and ones-column row-sum both
                                    # scale by e^-5), keeps exp in fp16 range
                                    hi_col = jc * P + 256 if ABLATE_EXPQ else 1024
                                    nc.scalar.activation(
                                        et[:, jc * P:hi_col], ps[:, jc * P:hi_col],
                                        AF.Exp, bias=neg5_t[:])
                                    # zero att where j > i on the diagonal
                                    nc.gpsimd.affine_select(
                                        out=et[:, jc * P:(jc + 1) * P],
                                        in_=et[:, jc * P:(jc + 1) * P],
                                        compare_op=OP.is_ge,
                                        fill=0.0, base=0,
                                        pattern=[[1, P]],
                                        channel_multiplier=-1)
                                    tiles[(h, jc)] = (et, 0)
                            else:
                                # both heads share one [128, 2, 512] tile
                                # (head hi in bank hi); single 2D-AP exp
                                e0 = (jc - 4) * P
                                ps = psp.tile([P, 1024], f32, tag="ps")
                                ps2 = ps[:].rearrange("p (t x) -> p t x", t=2)
                                for hi, h in enumerate(heads):
                                    hb = 64 * (h % 2)
                                    nc.tensor.matmul(
                                        ps2[:, hi, e0:512],
                                        vT[hb:hb + KD, hc, jc * P:(jc + 1) * P],
                                        kT[hb:hb + KD, hc, 512 + e0:1024],
                                        start=True, stop=True)
                                et = expp.tile([P, 1024], f16, tag="exp")
                                et2 = et[:].rearrange("p (t x) -> p t x", t=2)
                                hi2 = min(e0 + 128, 512) if ABLATE_EXPQ else 512
                                nc.scalar.activation(
                                    et2[:, :, e0:hi2], ps2[:, :, e0:hi2],
                                    AF.Exp, bias=neg5_t[:])
                                for hi, h in enumerate(heads):
                                    nc.gpsimd.affine_select(
                                        out=et2[:, hi, e0:e0 + P],
                                        in_=et2[:, hi, e0:e0 + P],
                                        compare_op=OP.is_ge,
                                        fill=0.0, base=0,
                                        pattern=[[1, P]],
                                        channel_multiplier=-1)
                                    tiles[(h, jc)] = (et, 512 * hi - 512)
                        exp_store[hpair] = tiles

                def emit_attend(S, x_new, x_newT, hpair):
                        q_sb = S["q"]
                        heads = (2 * hpair, 2 * hpair + 1)
                        tiles = S["store"].pop(hpair)
                        tp = None
                        for b in range(LB):
                            # both heads of the pair accumulate into one
                            # psum bank: head hi at cols [65*hi, 65*hi+65)
                            # (batching two b-blocks per bank measured WORSE,
                            # 431 vs 427us: normalize then waits both chains)
                            pa_ps = pa.tile([P, 130], f32, tag="pa")
                            jcs = ((0, b) if b else (0,)) if ABLATE_ATT \
                                else range(b + 1)
                            for hi, h in enumerate(heads):
                                for jc in jcs:
                                    et, iofs = tiles[(h, jc)]
                                    lc0 = iofs + b * P
                                    nc.tensor.matmul(
                                        pa_ps[:, 65 * hi:65 * hi + 65],
                                        et[:, lc0:lc0 + P],
                                        q_sb[:, jc, h, :],
                                        start=(jc == 0), stop=(jc == b))
                            pa2 = pa_ps[:].rearrange("p (h x) -> p h x", h=2)
                            rc = sm.tile([P, 2], f32, tag="rc")
                            nc.vector.reciprocal(rc[:], pa2[:, :, 64])
                            # x_new[:, b, heads] = att_u @ q * recip (recip
                            # broadcast 64-wide per head via 0-stride read)
                            xdst = x_new[:, b, :].rearrange(
                                "p (h x) -> p h x", h=H)[:, heads[0]:heads[0] + 2, :]
                            nc.vector.tensor_tensor(
                                xdst, pa2[:, :, 0:64],
                                rc[:, :, None].to_broadcast((P, 2, 64)),
                                OP.mult)
                        # this pair's x_newT column chunk, transposed as soon
                        # as its x_new columns land, so mlp1 can start right
                        # after the last attend. The LAST pair's drains gate
                        # every mlp1 psum; they go on ACT (its exps are done)
                        # instead of queueing behind DVE's attend normalizes.
                        for g in range(2):
                            tp = pp.tile([P, 512], f16, tag="pp")
                            for k in range(4):
                                b = 4 * g + k
                                nc.tensor.transpose(
                                    tp[:, k * P:(k + 1) * P],
                                    x_new[:, b, hpair * P:(hpair + 1) * P],
                                    ident[:])
                            if hpair == H // 2 - 1:
                                nc.scalar.copy(
                                    x_newT[:, hpair, g * 512:(g + 1) * 512],
                                    tp[:])
                            else:
                                nc.vector.tensor_copy(
                                    x_newT[:, hpair, g * 512:(g + 1) * 512],
                                    tp[:])

                # layer 0's first score pair is part of its head (later
                # layers get sc0 from the previous layer's tail)
                mark("L0.sc0")
                emit_scores(S, 0)

                # ---- layers ----
                for li in range(NL):
                    w1_t, w2_t = W["w1"], W["w2"]
                    x_new = natp.tile([P, LB, F], f16, tag="nat")
                    x_newT = bigT.tile([P, FC, L], f16, tag="bigT")

                    # interleave: kv chunk p feeds scores(p); LOOKAHEAD pairs
                    # of scores run ahead of the attend consuming them, so PE
                    # fills its exp-wait gaps with the next pairs' score
                    # matmuls and ACT stays saturated (kv chunk 0, q and sc0
                    # were emitted in the previous layer's tail)
                    for hp in range(-(LOOKAHEAD - 1), H // 2):
                        nxt = hp + LOOKAHEAD
                        if nxt < H // 2:
                            mark(f"L{li}.kv{nxt}")
                            emit_kv(W, S, xT, nxt, 0)
                            emit_kv(W, S, xT, nxt, 1)
                            mark(f"L{li}.sc{nxt}")
                            emit_scores(S, nxt)
                        if hp >= 0:
                            mark(f"L{li}.at{hp}")
                            emit_attend(S, x_new, x_newT, hp)

                    # MLP + layer tail, software-pipelined in l-halves: MLP2,
                    # LN, transposes, and the next layer's kv0/q matmuls for
                    # blocks 0-3 all start right after MLP1's first half
                    # instead of waiting for the whole MLP1.
                    h1T = bigT.tile([P, FC, L], f16, tag="bigT")
                    if use_b2:
                        b2_b = bcast_row(b2[li], f"b2_{li}")
                    if use_gamma:
                        gamma_b = bcast_row(gamma[li], f"g_{li}")
                    if use_beta:
                        beta_b = bcast_row(beta[li], f"be_{li}")
                    y = natp.tile([P, LB, F], f16, tag="nat")
                    mv8 = sm.tile([P, LB, 2], f32, tag="mv8")
                    rstd8 = sm.tile([P, LB], f32, tag="rs8")
                    xT_next = bigT.tile([P, FC, L], f16, tag="bigT")
                    if li + 1 < NL:
                        Wn = load_weights(li + 1)
                        Sn = new_attn_state()
                    else:
                        mark("unembed")
                        wo = []
                        for vc in range(2):
                            wt = wp.tile([P, FC, 512], f16, tag="w")
                            nc.sync.dma_start(
                                wt[:],
                                Wout[:, vc * 512:(vc + 1) * 512]
                                .rearrange("(c p) o -> p c o", p=P))
                            wo.append(wt)

                    def emit_mlp1(lc):
                        # h1T = relu(W1^T x_newT + b1); relu on ACT (its exp
                        # queue has drained by now, DVE is busier)
                        for oc in range(FC):
                            ps = pp.tile([P, 512], f32, tag="pp")
                            for fc in range(FC):
                                nc.tensor.matmul(
                                    ps[:],
                                    w1_t[:, fc, oc * P:(oc + 1) * P],
                                    x_newT[:, fc, lc * 512:(lc + 1) * 512],
                                    start=(fc == 0), stop=(fc == FC - 1))
                            nc.scalar.activation(
                                h1T[:, oc, lc * 512:(lc + 1) * 512],
                                ps[:], AF.Relu,
                                bias=b1_sb[:, li, oc:oc + 1] if use_b1 else 0.0)

                    def emit_mlp2_ln(b):
                        ps = pp.tile([P, 512], f32, tag="pp")
                        for fc in range(FC):
                            nc.tensor.matmul(
                                ps[:],
                                h1T[:, fc, b * P:(b + 1) * P],
                                w2_t[:, fc, :],
                                start=(fc == 0), stop=(fc == FC - 1))
                        t = y[:, b, :]
                        # residual add frees the psum slot quickly; stats and
                        # apply then run on the 2-byte sbuf copy (DVE 2x mode)
                        nc.vector.tensor_add(t, ps[:], x_nat[:, b, :])
                        if use_b2:
                            nc.vector.tensor_add(t, t, b2_b[:])
                        if ABLATE_LN:
                            return
                        st = sm.tile([P, 6], f32, tag="st")
                        nc.vector.bn_stats(st[:], t)
                        nc.vector.bn_aggr(mv8[:, b, :], st[:])
                        if b % 2 == 0:
                            return
                        # rstd = exp(-0.5*ln(var+eps)), batched per chunk
                        # PAIR: halves the ACT instruction count ahead of the
                        # next layer's sc0 exps, and the transposes gate on
                        # apply(b3)/apply(b7) whose timing is unchanged
                        # (rstd(b2,b3) still lands right after aggr(b3))
                        nc.scalar.activation(
                            rstd8[:, b - 1:b + 1], mv8[:, b - 1:b + 1, 1],
                            AF.Ln, bias=eps_t[:])
                        nc.scalar.activation(
                            rstd8[:, b - 1:b + 1], rstd8[:, b - 1:b + 1],
                            AF.Exp, scale=-0.5)
                        for bb in (b - 1, b):
                            tb = y[:, bb, :]
                            nc.vector.tensor_scalar(
                                tb, tb, mv8[:, bb, 0:1], rstd8[:, bb:bb + 1],
                                op0=OP.subtract, op1=OP.mult)
                            if use_gamma:
                                nc.vector.tensor_mul(tb, tb, gamma_b[:])
                            if use_beta:
                                nc.vector.tensor_add(tb, tb, beta_b[:])

                    # (a finer half-pipelined variant — mlp2/LN/tail per
                    # l-half right after each mlp1 half — measured WORSE,
                    # 487us vs 429us: the tail's ACT copies land between the
                    # two relu halves and delay mlp2. Keep the coarse order.)
                    mark(f"L{li}.mlp1")
                    for lc in range(2):
                        emit_mlp1(lc)
                    mark(f"L{li}.mlp2")
                    for b in range(LB):
                        emit_mlp2_ln(b)
                    mark(f"L{li}.xT")
                    if li + 1 < NL:
                        # tail interleave: the next layer's kv chunk 0 / q /
                        # sc0 matmuls fill the PE while DVE/ACT trickle
                        # through the LN chain, and keep the PE p-state warm
                        # into the next layer
                        transpose_b(y, xT_next, range(0, 4))
                        emit_kv(Wn, Sn, xT_next, 0, 0)
                        emit_q(Wn, Sn, xT_next, range(0, 4))
                        transpose_b(y, xT_next, range(4, 8))
                        emit_kv(Wn, Sn, xT_next, 0, 1)
                        emit_q(Wn, Sn, xT_next, range(4, 8))
                        emit_scores(Sn, 0)
                    else:
                        mark("unembed2")
                        transpose_b(y, xT_next, [0])
                        for b in range(LB):
                            if b + 1 < LB:
                                transpose_b(y, xT_next, [b + 1])
                            for vc in range(2):
                                    ps = pp.tile([P, 512], f32, tag="pp")
                                    for fc in range(FC):
                                        nc.tensor.matmul(
                                            ps[:],
                                            xT_next[:, fc, b * P:(b + 1) * P],
                                            wo[vc][:, fc, :],
                                            start=(fc == 0),
                                            stop=(fc == FC - 1))
                                    ot = outp.tile([P, 512], f32, tag="o")
                                    if use_bout:
                                        nc.vector.tensor_add(
                                            ot[:], ps[:],
                                            bout_b[:, vc * 512:(vc + 1) * 512])
                                    elif (b * 2 + vc) % 2 == 0:
                                        nc.scalar.copy(ot[:], ps[:])
                                    else:
                                        nc.vector.tensor_copy(ot[:], ps[:])
                                    if not ABLATE_OUT:
                                        nc.sync.dma_start(
                                            out[b * P:(b + 1) * P,
                                                vc * 512:(vc + 1) * 512],
                                            ot[:])

                    x_nat = y
                    if li + 1 < NL:
                        W, S, xT = Wn, Sn, xT_next
    nc.compile()
    return nc


def _get_nc(flags, repeat=1):
    key = (flags, repeat, PSUM_CFG, EXPP_BUFS, WP_BUFS, LOOKAHEAD,
           ABLATE_EXPQ, ABLATE_ATT, ABLATE_SC, ABLATE_LN, ABLATE_OUT)
    if key not in _NC_CACHE:
        _NC_CACHE[key] = _build(flags, repeat)
    return _NC_CACHE[key]


def _param_np_dtypes(nc):
    dt = {}
    for alloc in nc.m.functions[0].allocations:
        if isinstance(alloc, mybir.MemoryLocationSet) and alloc.kind == "ExternalInput":
            dt[alloc.memorylocations[0].name] = mybir.dt.np(alloc.dtype)
    return dt


def make_runner(flags, in_maps, repeat=1):
    """Build a reusable jitted SPMD runner with device-resident inputs.

    Returns (run, split_outputs) where run() executes the kernel once on all
    8 cores and blocks; used by test.py for timing without per-call
    host->device input transfer. Inputs are converted to each DRAM
    parameter's declared dtype (fp16 weights) host-side.
    """
    import jax
    from jax.sharding import Mesh, PartitionSpec, NamedSharding
    from concourse import bass2jax, mybir as _mybir

    bass2jax.install_neuronx_cc_hook()
    nc = _get_nc(flags, repeat)
    pdt = _param_np_dtypes(nc)
    partition_name = (nc.partition_id_tensor.name if nc.partition_id_tensor
                      else None)
    in_names, out_names, out_avals, zero_outs = [], [], [], []
    for alloc in nc.m.functions[0].allocations:
        if not isinstance(alloc, _mybir.MemoryLocationSet):
            continue
        name = alloc.memorylocations[0].name
        if alloc.kind == "ExternalInput":
            if name != partition_name:
                in_names.append(name)
        elif alloc.kind == "ExternalOutput":
            shape = tuple(alloc.tensor_shape)
            dtype = _mybir.dt.np(alloc.dtype)
            out_names.append(name)
            out_avals.append(jax.core.ShapedArray(shape, dtype))
            zero_outs.append(np.zeros(shape, dtype))
    n_params = len(in_names)
    n_outs = len(out_avals)
    all_names = in_names + out_names + ([partition_name] if partition_name else [])

    def _body(*args):
        operands = list(args)
        if partition_name is not None:
            operands.append(bass2jax.partition_id_tensor())
        outs = bass2jax._bass_exec_p.bind(
            *operands,
            out_avals=tuple(out_avals),
            in_names=tuple(all_names),
            out_names=tuple(out_names),
            lowering_input_output_aliases=(),
            sim_require_finite=True,
            sim_require_nnan=True,
            nc=nc,
        )
        return tuple(outs)

    from jax.experimental.shard_map import shard_map
    devices = jax.devices()[:NCORES]
    mesh = Mesh(np.asarray(devices), ("core",))
    in_specs = (PartitionSpec("core"),) * (n_params + n_outs)
    out_specs = (PartitionSpec("core"),) * n_outs
    sharded = jax.jit(
        shard_map(_body, mesh=mesh, in_specs=in_specs, out_specs=out_specs,
                  check_rep=False),
        keep_unused=True,
    )

    def conv(c, nm):
        return np.ascontiguousarray(np.asarray(in_maps[c][nm]), dtype=pdt[nm])

    concat_in = [
        np.concatenate([conv(c, nm)[None] for c in range(NCORES)],
                       axis=0).reshape(NCORES * conv(0, nm).shape[0],
                                       *conv(0, nm).shape[1:])
        for nm in in_names
    ]
    sh = NamedSharding(mesh, PartitionSpec("core"))
    dev_in = [jax.device_put(x, sh) for x in concat_in]
    dev_zeros = [
        jax.device_put(np.zeros((NCORES * z.shape[0], *z.shape[1:]), z.dtype), sh)
        for z in zero_outs
    ]

    def run():
        outs = sharded(*dev_in, *dev_zeros)
        jax.block_until_ready(outs)
        return outs

    def split(outs):
        return [
            {nm: np.asarray(outs[i]).reshape(NCORES, *out_avals[i].shape)[c]
             for i, nm in enumerate(out_names)}
            for c in range(NCORES)
        ]

    return run, split


def kernel(**inputs) -> np.ndarray:
    tokens = np.asarray(inputs["tokens"])
    args = {}
    for k, v in inputs.items():
        if k == "tokens":
            continue
        dt = np.float16 if k in _F16_PARAMS else np.float32
        args[k] = np.ascontiguousarray(np.asarray(v, dtype=np.float32), dtype=dt)
    f32args = {k: np.asarray(inputs[k], dtype=np.float32)
               for k in ("b1", "b2", "gamma", "beta", "bout")}
    flags = (
        bool(np.any(f32args["b1"])),
        bool(np.any(f32args["b2"])),
        bool(np.any(f32args["gamma"] != 1.0)),
        bool(np.any(f32args["beta"])),
        bool(np.any(f32args["bout"])),
    )
    nc = _get_nc(flags)
    tok32 = np.ascontiguousarray(tokens.astype(np.int32))
    in_maps = [dict(args, tokens=tok32[c]) for c in range(NCORES)]
    res = run_bass_kernel_spmd(nc, in_maps, list(range(NCORES)))
    return np.stack([res.results[c]["out"] for c in range(NCORES)], axis=0)


if __name__ == "__main__":
    rng = np.random.default_rng(0)
    toy = {
        "tokens": rng.integers(0, V, size=(N, L)),
        "embed": rng.standard_normal((V, F)).astype(np.float32) * 0.02,
        "Wq": rng.standard_normal((NL, F, HQ)).astype(np.float32) * 0.02,
        "Wk": rng.standard_normal((NL, F, H * KD)).astype(np.float32) * 0.02,
        "Wv": rng.standard_normal((NL, F, H * KD)).astype(np.float32) * 0.02,
        "W1": rng.standard_normal((NL, HQ, F)).astype(np.float32) * 0.02,
        "b1": np.zeros((NL, F), np.float32),
        "W2": rng.standard_normal((NL, F, F)).astype(np.float32) * 0.02,
        "b2": np.zeros((NL, F), np.float32),
        "gamma": np.ones((NL, F), np.float32),
        "beta": np.zeros((NL, F), np.float32),
        "Wout": rng.standard_normal((F, V)).astype(np.float32) * 0.02,
        "bout": np.zeros((V,), np.float32),
    }
    o = kernel(**toy)
    print("out:", o.shape, o.dtype, float(np.abs(o).max()))
